# revision 14
# baseline (speedup 1.0000x reference)
"""LoRA-MLP kernel for 8x TRN2 NeuronCores (SPMD data-parallel over batch).

Math (per batch b):
    z1 = (x @ v) / IN            [F, R]
    z  = (z1 @ u.T) / R          [F, OUT]
    y  = gelu(x @ W.T + fc_bias + z + b)

Device formulation (per core, 4 batches), all PSUM-accumulated per f-tile:
    psum[f, o] = ones[1,f].T @ bias[1,o]          (K=1: fc_bias + b)
               + sum_k xT[k][:, f].T @ WT[k][:, o]  (8 K-tiles of 128)
               + z1T[:, f].T @ uT[:, o]             (K=16 LoRA)
    out = gelu(psum)   (ScalarE, PSUM -> SBUF fp32)
    z1T[r, f] = sum_k vs[k].T @ xT[k]  on PE, copied PSUM->SBUF via ScalarE.

All matmul operands bf16 (host-cast/laid out); fp32 accumulation in PSUM.
Sync-wait budget note: this codegen allows roughly one semaphore wait per
compute instruction (2 for DMA), so pools are sized for zero slot reuse and
each producer/consumer pair crosses engines exactly once.
"""

import sys

for _p in ("/opt/trn_rl_repo", "/opt/pypackages"):
    if _p not in sys.path:
        sys.path.append(_p)

import numpy as np
import ml_dtypes

B, F, IN, OUT, R = 32, 512, 1024, 1024, 16
NCORES = 8
BPC = B // NCORES  # batches per core = 4
KT = IN // 128  # 8 K-tiles
FT = F // 128  # 4 F-tiles per batch
BF16 = ml_dtypes.bfloat16

_COMPILED = {}


def _build_nc():
    import concourse.tile as tile
    from concourse import bacc, mybir

    # Bacc (not raw Bass): its compile() runs generate_event_semaphores,
    # which splits multi-sem waits — walrus codegen allows only one sync
    # wait per instruction.
    nc = bacc.Bacc(None)
    bf = mybir.dt.bfloat16
    f32 = mybir.dt.float32

    xt = nc.declare_dram_parameter("xt", [BPC, 128, KT, F], bf, isOutput=False)
    wt = nc.declare_dram_parameter("wt", [128, KT, OUT], bf, isOutput=False)
    vs = nc.declare_dram_parameter("vs", [BPC, 128, KT, R], bf, isOutput=False)
    ut = nc.declare_dram_parameter("ut", [BPC, R, OUT], bf, isOutput=False)
    bias = nc.declare_dram_parameter("bias", [BPC, 1, OUT], bf, isOutput=False)
    ones = nc.declare_dram_parameter("ones", [1, 128], bf, isOutput=False)
    y = nc.declare_dram_parameter("y", [BPC, FT, 128, OUT], f32, isOutput=True)

    GELU = mybir.ActivationFunctionType.Gelu

    with tile.TileContext(nc) as tc:
        with (
            tc.tile_pool(name="const", bufs=1) as const_pool,
            tc.tile_pool(name="xin", bufs=BPC) as xin_pool,
            tc.tile_pool(name="small", bufs=BPC) as small_pool,
            tc.tile_pool(name="out", bufs=2 * FT * BPC) as out_pool,
            tc.tile_pool(name="psum", bufs=4, space="PSUM") as psum_pool,
            tc.tile_pool(name="zpsum", bufs=2, space="PSUM") as zpsum_pool,
        ):
            wt_sb = const_pool.tile([128, KT, OUT], bf)
            nc.sync.dma_start(out=wt_sb[:], in_=wt[:])
            ones_sb = const_pool.tile([1, 128], bf)
            nc.sync.dma_start(out=ones_sb[:], in_=ones[:])

            z1_tiles = [
                const_pool.tile([R, F], bf, name=f"z1_{i}", tag=f"z1_{i}")
                for i in range(BPC)
            ]

            for b in range(BPC):
                xt_sb = xin_pool.tile([128, KT, F], bf, tag="xt")
                nc.sync.dma_start(out=xt_sb[:], in_=xt[b])
                vs_sb = small_pool.tile([128, KT, R], bf, tag="vs")
                nc.sync.dma_start(out=vs_sb[:], in_=vs[b])
                ut_sb = small_pool.tile([R, OUT], bf, tag="ut")
                nc.sync.dma_start(out=ut_sb[:], in_=ut[b])
                bias_sb = small_pool.tile([1, OUT], bf, tag="bias")
                nc.sync.dma_start(out=bias_sb[:], in_=bias[b])

                # Stage 1: z1T[r, f] = sum_k vs[k].T @ xT[k]  -> [16, F] PSUM
                z1_ps = zpsum_pool.tile([R, F], f32, tag="z1ps")
                for k in range(KT):
                    nc.tensor.matmul(
                        z1_ps[:],
                        lhsT=vs_sb[:, k, :],
                        rhs=xt_sb[:, k, :],
                        start=(k == 0),
                        stop=(k == KT - 1),
                    )
                z1_sb = z1_tiles[b]
                nc.scalar.copy(z1_sb[:], z1_ps[:])

                # Stage 2: bias + main matmul + LoRA, accumulated in PSUM.
                for ft in range(FT):
                    fsl = slice(ft * 128, (ft + 1) * 128)
                    ps0 = psum_pool.tile([128, 512], f32, tag="ps")
                    ps1 = psum_pool.tile([128, 512], f32, tag="ps")
                    nc.tensor.matmul(
                        ps0[:], lhsT=ones_sb[:], rhs=bias_sb[:, 0:512],
                        start=True, stop=False,
                    )
                    nc.tensor.matmul(
                        ps1[:], lhsT=ones_sb[:], rhs=bias_sb[:, 512:1024],
                        start=True, stop=False,
                    )
                    for k in range(KT):
                        lhsT = xt_sb[:, k, fsl]
                        nc.tensor.matmul(
                            ps0[:], lhsT=lhsT, rhs=wt_sb[:, k, 0:512],
                            start=False, stop=False,
                        )
                        nc.tensor.matmul(
                            ps1[:], lhsT=lhsT, rhs=wt_sb[:, k, 512:1024],
                            start=False, stop=False,
                        )
                    nc.tensor.matmul(
                        ps0[:], lhsT=z1_sb[:, fsl], rhs=ut_sb[:, 0:512],
                        start=False, stop=True,
                    )
                    nc.tensor.matmul(
                        ps1[:], lhsT=z1_sb[:, fsl], rhs=ut_sb[:, 512:1024],
                        start=False, stop=True,
                    )
                    o0 = out_pool.tile([128, 512], f32, tag="o")
                    o1 = out_pool.tile([128, 512], f32, tag="o")
                    nc.scalar.activation(o0[:], ps0[:], GELU)
                    nc.scalar.activation(o1[:], ps1[:], GELU)
                    nc.sync.dma_start(out=y[b, ft, :, 0:512], in_=o0[:])
                    nc.sync.dma_start(out=y[b, ft, :, 512:1024], in_=o1[:])
    nc.finalize()
    return nc


def _shard_inputs(x, u, v, b, W, fc_bias):
    """Build per-core device input dicts (host-side layout + bf16 cast)."""
    # xt[c][bb, p, k, f] = x[4c+bb, f, 128k+p]
    xt = np.ascontiguousarray(
        x.reshape(B, F, KT, 128).transpose(0, 3, 2, 1)
    ).astype(BF16)
    # wt[p, k, o] = W[o, 128k+p]
    wt = np.ascontiguousarray(W.reshape(OUT, KT, 128).transpose(2, 1, 0)).astype(BF16)
    # vs[bb, p, k, r] = v[bb, 0, 128k+p, r] / (IN*R)
    vs = np.ascontiguousarray(
        (v[:, 0] / float(IN * R)).reshape(B, KT, 128, R).transpose(0, 2, 1, 3)
    ).astype(BF16)
    # ut[bb, r, o] = u[bb, 0, o, r]
    ut = np.ascontiguousarray(u[:, 0].transpose(0, 2, 1)).astype(BF16)
    bias = (fc_bias[None, None, :] + b).astype(BF16)  # [B, 1, OUT]

    in_maps = []
    for c in range(NCORES):
        s = slice(c * BPC, (c + 1) * BPC)
        in_maps.append(
            {
                "xt": xt[s],
                "wt": wt,
                "vs": vs[s],
                "ut": ut[s],
                "bias": np.ascontiguousarray(bias[s]),
                "ones": np.ones((1, 128), dtype=BF16),
            }
        )
    return in_maps


def _run(in_maps, trace=False, **kw):
    from concourse import bass_utils

    key = "nc"
    if key not in _COMPILED:
        _COMPILED[key] = _build_nc()
    nc = _COMPILED[key]
    res = bass_utils.run_bass_kernel_spmd(
        nc, in_maps, list(range(NCORES)), trace=trace, **kw
    )
    return res


def kernel(x, u, v, b, W, fc_bias):
    x = np.asarray(x, dtype=np.float32)
    u = np.asarray(u, dtype=np.float32)
    v = np.asarray(v, dtype=np.float32)
    b = np.asarray(b, dtype=np.float32)
    W = np.asarray(W, dtype=np.float32)
    fc_bias = np.asarray(fc_bias, dtype=np.float32)

    in_maps = _shard_inputs(x, u, v, b, W, fc_bias)
    res = _run(in_maps, trace=False)
    outs = [r["y"].reshape(BPC, F, OUT) for r in res.results]
    return np.concatenate(outs, axis=0).astype(np.float32)
